# revision 33
# baseline (speedup 1.0000x reference)
"""Trainium2 Bass kernel for grouped multi-head attention (nn_Attention_8263517077742).

Reference computation (per batch b, group g, with x [2048, 512]):
  xn   = x / max(||x||_2, eps) * sqrt(512)        (rmsnorm over feature dim)
  q    = (xn * gamma_q) @ wq[g].T                 -> 8 heads of 64
  k,v  = (xn * gamma_c) @ wkv[g].T                -> 8 heads of 64
  null k/v prepended along key sequence; scores masked by mask[b]; softmax;
  merged heads projected by wout[g].

Sharding: 8 cores = 4 (b,g) instances x 2 query-sequence halves. Masked keys
contribute nothing (exp -> 0) and the mask is an input, so the host packs only
the valid key rows (~1010-1040 of 2048) plus one null-kv slot into 9 tiles of
128; pad slots get a -1e30 exp bias. Outputs are disjoint: no collectives.

The PE on this part runs at ~1.2 GHz with a per-matmul weight load, so the
design minimizes PE cycles and keeps the PE streaming:
  - the host supplies x both natural (for the row norms) and pre-transposed
    (bf16 xqt/xkt), so there are no on-chip transposes at all
  - rmsnorm is folded away: projections consume unnormalized xT, and the
    1/||x|| scales are applied as (a) the per-partition `scale` operand of the
    softmax exp for the key side, (b) a broadcast multiply at q staging, (c) a
    per-tile scalar multiply at v staging; gamma, sqrt(D) and the attention
    scale are folded into the weights host-side
  - everything on the PE is bf16 (fp32 PSUM accumulation); fp8 was measured
    and rejected: DoubleRow scores cost 1.65e-2 of the 2e-2 error budget
  - scores are computed transposed (sT [nk, nq]) in 512-query chunks, two
    t-steps ahead of the exp (ScalarE, mask bias + key-norm scale fused)
  - the null k/v pair occupies packed key slot 1120: its k column / v row are
    patched in after the projections, so no separate rank-1 update exists
  - softmax denominators ride as a 65th row of the AV matmul (ones column in
    v); normalization = reciprocal + partition-broadcast + multiply in bf16
  - merged heads are head-pair packed so the output projection contracts over
    full 128 partitions; its per-pair partial sums stream into the attention
    loop as PE fillers and accumulate in SBUF, with per-tile output DMAs
  - k projections lead the prologue (their staging has no alpha dependency)
    interleaved with the norm side-chain; weights/constants load outside the
    timing loop
"""

import sys
from contextlib import ExitStack

import numpy as np
import ml_dtypes

if "/opt/trn_rl_repo" not in sys.path:
    sys.path.insert(0, "/opt/trn_rl_repo")

import concourse.bass as bass  # noqa: E402
import concourse.mybir as mybir  # noqa: E402
from concourse import bacc  # noqa: E402
from concourse.tile import TileContext  # noqa: E402

P = 128
D = 512           # feature dim
E = 512           # inner dim (8 heads x 64)
NQ = 1024         # queries per core
H = 8
DH = 64
NK = 1152         # packed key slots per core (valid keys + pads + null at 1151)
NKT = NK // P     # 9 key tiles
QT = NQ // P      # 8 query tiles
ET = E // P       # 4 e-tiles
DT = D // P       # 4 d-tiles
VEXT = H * (DH + 1)   # 520: per-head v columns + ones column
NULL_SLOT = NK - 32   # 1120: null-kv key slot (partition 96 of the last tile)
F32 = mybir.dt.float32
BF16 = mybir.dt.bfloat16
FP8 = mybir.dt.float8e4
FP8_SCORES = False  # quantize q/k to fp8e4m3, score matmuls in DoubleRow mode

B, G = 2, 2


def build_nc(reps=1):
    nc = bacc.Bacc(
        trn_type="TRN2",
        target_bir_lowering=False,
        debug=False,
        enable_asserts=False,
        num_devices=8,
    )
    xq_ext = nc.declare_dram_parameter("xq", [NQ, D], BF16, isOutput=False)
    xk_ext = nc.declare_dram_parameter("xk", [NK, D], BF16, isOutput=False)
    xqt_ext = nc.declare_dram_parameter("xqt", [D, NQ], BF16, isOutput=False)
    xkt_ext = nc.declare_dram_parameter("xkt", [D, NK], BF16, isOutput=False)
    wq_ext = nc.declare_dram_parameter("wq_t", [D, E], BF16, isOutput=False)
    wk_ext = nc.declare_dram_parameter("wk_t", [D, E], BF16, isOutput=False)
    wv_ext = nc.declare_dram_parameter("wv_t", [D, E], BF16, isOutput=False)
    wo_ext = nc.declare_dram_parameter("wo_t", [E, D], BF16, isOutput=False)
    mb_ext = nc.declare_dram_parameter("maskbias", [P, NKT], F32, isOutput=False)
    nk_ext = nc.declare_dram_parameter("nullk", [P, ET],
                                       FP8 if FP8_SCORES else BF16, isOutput=False)
    nv_ext = nc.declare_dram_parameter("nullv_ext", [1, VEXT], BF16, isOutput=False)
    out_ext = nc.declare_dram_parameter("out", [NQ, D], BF16, isOutput=True)

    with TileContext(nc) as tc, ExitStack() as ctx:
        # ---- persistent SBUF tiles (weights/consts loaded outside the loop) ----
        persist = ctx.enter_context(tc.tile_pool(name="persist", bufs=1))
        # transposed normalized x, dj-blocked single tiles
        xqT = persist.tile([P, DT * NQ], BF16, name="xqT", tag="xqT")
        xkT = persist.tile([P, DT * NK], BF16, name="xkT", tag="xkT")
        QKDT = FP8 if FP8_SCORES else BF16
        qT = [persist.tile([P, NQ], QKDT, name=f"qT{j}", tag=f"qT{j}") for j in range(ET)]
        kT = [persist.tile([P, NK], QKDT, name=f"kT{j}", tag=f"kT{j}") for j in range(ET)]
        if FP8_SCORES:
            # partition-folded DoubleRow operands: [32, (head, plane, n)]
            q8 = [persist.tile([32, 4 * NQ], FP8, name=f"q8{j}", tag=f"q8{j}")
                  for j in range(ET)]
            k8 = [persist.tile([32, 4 * NK], FP8, name=f"k8{j}", tag=f"k8{j}")
                  for j in range(ET)]
        v_ext = [persist.tile([P, VEXT], BF16, name=f"vx{i}", tag=f"vx{i}") for i in range(NKT)]
        mgT = [persist.tile([P, NQ], BF16, name=f"mg{p}", tag=f"mg{p}") for p in range(H // 2)]
        wq_sb = [persist.tile([P, E], BF16, name=f"wq{j}", tag=f"wq{j}") for j in range(DT)]
        wk_sb = [persist.tile([P, E], BF16, name=f"wk{j}", tag=f"wk{j}") for j in range(DT)]
        wv_sb = [persist.tile([P, E], BF16, name=f"wv{j}", tag=f"wv{j}") for j in range(DT)]
        wo_sb = [persist.tile([P, D], BF16, name=f"wo{p}", tag=f"wo{p}") for p in range(H // 2)]
        mb_sb = persist.tile([P, NKT], F32, name="mb", tag="mb")
        aq_sb = persist.tile([P, QT], F32, name="aq", tag="aq")
        ak_sb = persist.tile([P, NKT], F32, name="ak", tag="ak")
        sk_sb = persist.tile([P, NKT], F32, name="sk", tag="sk")
        arow = persist.tile([1, NQ], F32, name="arow", tag="arow")
        abcq = persist.tile([P, NQ], F32, name="abcq", tag="abcq")
        nk_sb = persist.tile([P, ET], FP8 if FP8_SCORES else BF16, name="nk", tag="nk")
        nv_sb = persist.tile([1, VEXT], BF16, name="nv", tag="nv")
        onesc = persist.tile([P, H], BF16, name="onesc", tag="onesc")
        osb = persist.tile([P, QT * D], BF16, name="osb", tag="osb")

        # ---- PSUM pools (2 + 4 + 2 = 8 banks) ----
        ppsum = ctx.enter_context(tc.tile_pool(name="ppsum", bufs=2, space="PSUM"))
        sps = ctx.enter_context(tc.tile_pool(name="sps", bufs=2, space="PSUM"))
        avps = ctx.enter_context(tc.tile_pool(name="avps", bufs=1, space="PSUM"))

        # ---- working SBUF pools ----
        xpool = ctx.enter_context(tc.tile_pool(name="xpool", bufs=6))
        xnpool = ctx.enter_context(tc.tile_pool(name="xnpool", bufs=3))
        ppool = ctx.enter_context(tc.tile_pool(name="ppool", bufs=3))
        rpool = ctx.enter_context(tc.tile_pool(name="rpool", bufs=2))

        nc.gpsimd.memset(onesc[:, :], 1.0)
        nc.sync.dma_start(out=mb_sb[:, :], in_=mb_ext[:, :])
        nc.sync.dma_start(out=nk_sb[:, :], in_=nk_ext[:, :])
        nc.sync.dma_start(out=nv_sb[:, :], in_=nv_ext[:, :])
        for j in range(DT):
            nc.sync.dma_start(out=wq_sb[j][:, :], in_=wq_ext[j * P:(j + 1) * P, :])
            nc.sync.dma_start(out=wk_sb[j][:, :], in_=wk_ext[j * P:(j + 1) * P, :])
            nc.sync.dma_start(out=wv_sb[j][:, :], in_=wv_ext[j * P:(j + 1) * P, :])
        for p in range(H // 2):
            nc.sync.dma_start(out=wo_sb[p][:, :], in_=wo_ext[p * P:(p + 1) * P, :])

        if reps > 1:
            ctx.enter_context(tc.For_i(
                0, reps, 1,
                hint_engines=(
                    mybir.EngineType.PE, mybir.EngineType.DVE,
                    mybir.EngineType.Activation, mybir.EngineType.SP,
                    mybir.EngineType.Pool,
                ),
            ))

        # per-rep DMAs: host-transposed x feeds the projections directly;
        # xkt first -- the k projection is the PE's first work of the rep
        for dj in range(DT):
            nc.sync.dma_start(out=xkT[:, dj * NK:(dj + 1) * NK],
                              in_=xkt_ext[dj * P:(dj + 1) * P, :])
        for dj in range(DT):
            nc.sync.dma_start(out=xqT[:, dj * NQ:(dj + 1) * NQ],
                              in_=xqt_ext[dj * P:(dj + 1) * P, :])
        xts = {}

        def dma_x(kind, i):
            xt = xpool.tile([P, D], BF16, name="x", tag="x", bufs=6)
            src = xq_ext if kind == "q" else xk_ext
            nc.sync.dma_start(out=xt[:, :], in_=src[i * P:(i + 1) * P, :])
            xts[(kind, i)] = xt

        for i in range(QT):
            dma_x("q", i)
        for i in range(NKT):
            dma_x("k", i)

        def norm_alpha(kind, i):
            """rmsnorm scale alpha[i] = 1/max(||x_row||, eps) for one tile."""
            xt = xts[(kind, i)]
            xsq = xnpool.tile([P, D], BF16, name="xsq", tag="xsq")
            ss = xnpool.tile([P, 1], F32, name="ss", tag="ss")
            nc.gpsimd.tensor_mul(xsq[:, :], xt[:, :], xt[:, :])
            nc.vector.tensor_reduce(
                ss[:, :], xsq[:, :], axis=mybir.AxisListType.X,
                op=mybir.AluOpType.add,
            )
            nrm = xnpool.tile([P, 1], F32, name="nrm", tag="nrm")
            nc.scalar.activation(
                nrm[:, :], ss[:, :], mybir.ActivationFunctionType.Sqrt,
            )
            nc.gpsimd.tensor_scalar_max(nrm[:, :], nrm[:, :], 1e-12)
            a_sb = aq_sb if kind == "q" else ak_sb
            nc.vector.reciprocal(a_sb[:, i:i + 1], nrm[:, :])

        def emit_qproj(c):
            """qT[:, c*512:(c+1)*512] for all 4 e-tiles."""
            for j in range(ET):
                pq = ppsum.tile([P, D], F32, name="pp", tag="pp")
                for dj in range(DT):
                    nc.tensor.matmul(
                        pq[:, :],
                        lhsT=wq_sb[dj][:, j * P:(j + 1) * P],
                        rhs=xqT[:, dj * NQ + c * 512:dj * NQ + (c + 1) * 512],
                        start=(dj == 0), stop=(dj == DT - 1),
                    )
                nc.vector.tensor_mul(qT[j][:, c * 512:(c + 1) * 512],
                                     pq[:, :], abcq[:, c * 512:(c + 1) * 512])
                if FP8_SCORES and c == 1:
                    for hh in range(2):
                        for two in range(2):
                            nc.sync.dma_start(
                                out=q8[j][:, (hh * 2 + two) * NQ:
                                          (hh * 2 + two + 1) * NQ],
                                in_=qT[j][hh * DH + two * 32:
                                          hh * DH + two * 32 + 32, :],
                            )

        KCH = [(0, 512), (512, 1024), (1024, NK)]   # k-proj column chunks

        def emit_kproj(j, c):
            lo, hi = KCH[c]
            pk = ppsum.tile([P, D], F32, name="pp", tag="pp")
            for dj in range(DT):
                nc.tensor.matmul(
                    pk[:, 0:hi - lo],
                    lhsT=wk_sb[dj][:, j * P:(j + 1) * P],
                    rhs=xkT[:, dj * NK + lo:dj * NK + hi],
                    start=(dj == 0), stop=(dj == DT - 1),
                )
            nc.vector.tensor_copy(kT[j][:, lo:hi], pk[:, 0:hi - lo])
            if c == len(KCH) - 1:
                # null-k column: its packed key slot bypasses the projection
                nc.gpsimd.tensor_copy(kT[j][:, NULL_SLOT:NULL_SLOT + 1],
                                      nk_sb[:, j:j + 1])
                if FP8_SCORES:
                    for hh in range(2):
                        for two in range(2):
                            nc.sync.dma_start(
                                out=k8[j][:, (hh * 2 + two) * NK:
                                          (hh * 2 + two + 1) * NK],
                                in_=kT[j][hh * DH + two * 32:
                                          hh * DH + two * 32 + 32, :],
                            )

        def emit_vproj(i):
            pv = ppsum.tile([P, D], F32, name="pp", tag="pp")
            for dj in range(DT):
                nc.tensor.matmul(
                    pv[:, :],
                    lhsT=xkT[:, dj * NK + i * P:dj * NK + (i + 1) * P],
                    rhs=wv_sb[dj][:, :],
                    start=(dj == 0), stop=(dj == DT - 1),
                )
            src = pv[:, :].rearrange("p (a d) -> p a d", a=H)
            dst = v_ext[i][:, :].rearrange("p (a r) -> p a r", a=H)
            nc.vector.tensor_scalar(dst[:, :, 0:DH], src[:, :, :],
                                    ak_sb[:, i:i + 1], None,
                                    op0=mybir.AluOpType.mult)
            nc.gpsimd.tensor_copy(dst[:, :, DH:DH + 1],
                                  onesc[:, :].rearrange("p (a r) -> p a r", a=H))
            if i == NKT - 1:
                # null-v row (+ its ones entry) at the null key slot
                p0 = NULL_SLOT % P
                nc.gpsimd.tensor_copy(v_ext[i][p0:p0 + 1, :], nv_sb[:, :])

        # ---- prologue ----
        # k projections lead: their staging copies have no alpha dependency,
        # so PE/DVE stream from the moment xkt lands; the norm side-chain
        # (Pool mul/reduce/max + ACT sqrt + DVE recip) fills in behind
        emit_kproj(0, 0)
        emit_kproj(0, 1)
        emit_kproj(0, 2)
        # interleave the remaining k-proj chunks with the norm side-chain so
        # neither DVE staging nor the alpha chain head-of-line blocks the PE
        norms = [("q", i) for i in range(QT)] + [("k", i) for i in range(NKT)]
        kchunks = [(j, c) for j in range(1, ET) for c in range(len(KCH))]
        nk_ratio = [1] * 6 + [2] * 6
        while norms or kchunks:
            if kchunks:
                j, c = kchunks.pop(0)
                emit_kproj(j, c)
            for _ in range(nk_ratio.pop(0) if nk_ratio else 2):
                if norms:
                    norm_alpha(*norms.pop(0))
        # query-side alpha: [128, QT] -> one [1, NQ] row -> partition bcast
        for t in range(QT):
            nc.sync.dma_start(out=arow[:, t * P:(t + 1) * P],
                              in_=aq_sb[:, t:t + 1])
        nc.gpsimd.partition_broadcast(abcq[:, :], arow[:, :])
        # key-side alpha becomes the exp scale; null slot must not be scaled
        nc.gpsimd.tensor_copy(sk_sb[:, :], ak_sb[:, :])
        nc.gpsimd.memset(sk_sb[NULL_SLOT % P:NULL_SLOT % P + 1,
                               NULL_SLOT // P:NULL_SLOT // P + 1], 1.0)

        emit_qproj(0)
        emit_qproj(1)
        for i in range(NKT):
            emit_vproj(i)
        # dummy exp: pulls the exp table-set load off the first score's path
        escr = xnpool.tile([1, 1], F32, name="escr", tag="escr")
        nc.scalar.activation(escr[:, :], mb_sb[0:1, 0:1],
                             mybir.ActivationFunctionType.Exp)

        # filler queue: streamed output-projection units join per head pair
        fillers = []

        # ---- attention: 9 key tiles x 8 heads, scores 2 t-steps ahead ----
        def emit_scores(h, t):
            j, hh = h // 2, h % 2
            st = sps.tile([P, NQ], F32, name="st", tag="st")
            if FP8_SCORES:
                kv_ = k8[j][:, :].rearrange("p (hh two n) -> p hh two n", hh=2, two=2)
                qv_ = q8[j][:, :].rearrange("p (hh two n) -> p hh two n", hh=2, two=2)
                for c in range(2):
                    nc.tensor.matmul(
                        st[:, c * 512:(c + 1) * 512],
                        lhsT=kv_[:, hh, :, t * P:(t + 1) * P],
                        rhs=qv_[:, hh, :, c * 512:(c + 1) * 512],
                        start=True, stop=True,
                        perf_mode=mybir.MatmulPerfMode.DoubleRow,
                    )
            else:
                off = DH * hh
                for c in range(2):
                    nc.tensor.matmul(
                        st[:, c * 512:(c + 1) * 512],
                        lhsT=kT[j][off:off + DH, t * P:(t + 1) * P],
                        rhs=qT[j][off:off + DH, c * 512:(c + 1) * 512],
                        start=True, stop=True,
                    )
            return st

        def emit_oproj(p, cq):
            po = ppsum.tile([P, D], F32, name="pp", tag="pp")
            nc.tensor.matmul(
                po[:, :],
                lhsT=mgT[p][:, cq * P:(cq + 1) * P],
                rhs=wo_sb[p][:, :],
                start=True, stop=True,
            )
            if p == 0:
                nc.vector.tensor_copy(osb[:, cq * D:(cq + 1) * D], po[:, :])
            else:
                nc.vector.tensor_add(osb[:, cq * D:(cq + 1) * D],
                                     osb[:, cq * D:(cq + 1) * D], po[:, :])
            if p == H // 2 - 1:
                nc.sync.dma_start(out=out_ext[cq * P:(cq + 1) * P, :],
                                  in_=osb[:, cq * D:(cq + 1) * D])

        pending = []      # queue of (h, t, [st chunks]) not yet exp'd
        av = None

        def emit_step():
            """exp + AV for the oldest pending score pair."""
            h, t, st = pending.pop(0)
            pt = ppool.tile([P, NQ], BF16, name="pt", tag="pt")
            nc.scalar.activation(
                pt[:, :], st[:, :], mybir.ActivationFunctionType.Exp,
                bias=mb_sb[:, t:t + 1], scale=sk_sb[:, t:t + 1],
            )
            for c in range(2):
                nc.tensor.matmul(
                    av[:, c * 512:(c + 1) * 512],
                    lhsT=v_ext[t][:, h * (DH + 1):(h + 1) * (DH + 1)],
                    rhs=pt[:, c * 512:(c + 1) * 512],
                    start=(t == 0), stop=(t == NKT - 1),
                )

        for h in range(H):
            av = avps.tile([DH + 1, NQ], F32, name="av", tag="av")
            if h == 0:
                pending.append((0, 0, emit_scores(0, 0)))
                pending.append((0, 1, emit_scores(0, 1)))
            for t in range(NKT):
                # keep scores 2 steps ahead of exp/AV
                if t + 2 < NKT:
                    pending.append((h, t + 2, emit_scores(h, t + 2)))
                elif h + 1 < H:
                    t2 = t + 2 - NKT
                    pending.append((h + 1, t2, emit_scores(h + 1, t2)))
                emit_step()
                if fillers:
                    fillers.pop()()
            # normalize head h: avc (ACT) frees the PSUM bank, then
            # recip (DVE) + broadcast (Pool) + merge mul (DVE), all bf16;
            # the last head runs in halves so pair-3 out-proj starts sooner
            halves = ((0, 512), (512, NQ)) if h == H - 1 else ((0, NQ),)
            avc = rpool.tile([DH + 1, NQ], BF16, name="avc", tag="avc")
            recip = rpool.tile([1, NQ], BF16, name="recip", tag="recip")
            rbc = rpool.tile([DH, NQ], BF16, name="rbc", tag="rbc")
            off = DH * (h % 2)
            for lo, hi in halves:
                nc.scalar.copy(avc[:, lo:hi], av[:, lo:hi])
                with nc.allow_low_precision(reason="bf16 softmax renorm; tol 2e-2"):
                    nc.vector.reciprocal(recip[:, lo:hi], avc[DH:DH + 1, lo:hi])
                nc.gpsimd.partition_broadcast(rbc[:, lo:hi], recip[:, lo:hi])
                nc.vector.tensor_mul(mgT[h // 2][off:off + DH, lo:hi],
                                     avc[0:DH, lo:hi], rbc[:, lo:hi])
                if h == H - 1:
                    for cq in range(lo // P, hi // P):
                        emit_oproj(h // 2, cq)
            if h % 2 == 1 and h < H - 1:
                pr = h // 2
                fillers[0:0] = [lambda p=pr, cq=cq: emit_oproj(p, cq)
                                for cq in reversed(range(QT))]

        # ---- drain remaining output-projection units (DMAs stream per cq) ----
        while fillers:
            fillers.pop()()

    nc.compile()
    return nc


_NC_CACHE = []


def get_nc():
    if not _NC_CACHE:
        _NC_CACHE.append(build_nc())
    return _NC_CACHE[0]


def make_in_maps(x, mask, gamma_q, gamma_c, wq, wkv, wout, null_kv):
    x = np.asarray(x, dtype=np.float32)
    mask = np.asarray(mask)
    gamma_q = np.asarray(gamma_q, dtype=np.float32)
    gamma_c = np.asarray(gamma_c, dtype=np.float32)
    wq = np.asarray(wq, dtype=np.float32)
    wkv = np.asarray(wkv, dtype=np.float32)
    wout = np.asarray(wout, dtype=np.float32)
    null_kv = np.asarray(null_kv, dtype=np.float32)
    bf16 = ml_dtypes.bfloat16

    sqD = np.float32(np.sqrt(D))
    scale = np.float32(DH ** -0.5)
    DI = E

    per_g = {}
    for g in range(G):
        wq_t = np.ascontiguousarray(
            (wq[g] * (gamma_q[g] * sqD * scale)[None, :]).T).astype(bf16)
        wk_t = np.ascontiguousarray(
            (wkv[g][:DI] * (gamma_c[g] * sqD)[None, :]).T).astype(bf16)
        wv_t = np.ascontiguousarray(
            (wkv[g][DI:] * (gamma_c[g] * sqD)[None, :]).T).astype(bf16)
        wo_t = np.ascontiguousarray(wout[g].T).astype(bf16)
        # null-k stacked e-major -> [128, 4] columns; null-v interleaved + ones
        nk_flat = null_kv[0, g].reshape(E)             # [H,1,DH] -> [512]
        nk_dt = ml_dtypes.float8_e4m3 if FP8_SCORES else bf16
        nk_cols = np.ascontiguousarray(nk_flat.reshape(ET, P).T).astype(nk_dt)
        nve = np.zeros((1, VEXT), np.float32)
        for h in range(H):
            nve[0, h * (DH + 1):h * (DH + 1) + DH] = null_kv[1, g, h, 0, :]
            nve[0, h * (DH + 1) + DH] = 1.0
        per_g[g] = (wq_t, wk_t, wv_t, wo_t, nk_cols, nve.astype(bf16))

    per_b = {}
    for b in range(B):
        idx = np.nonzero(mask[b])[0]
        nv = len(idx)
        assert nv <= NULL_SLOT, f"valid keys {nv} exceed packed capacity {NULL_SLOT}"
        bias = np.full(NK, np.float32(-1e30), np.float32)
        bias[:nv] = 0.0
        bias[NULL_SLOT] = 0.0               # null slot is always valid
        per_b[b] = (idx, np.ascontiguousarray(bias.reshape(NKT, P).T))

    in_maps = []
    for c in range(8):
        b, g, half = c // 4, (c // 2) % 2, c % 2
        wq_t, wk_t, wv_t, wo_t, nk_cols, nve = per_g[g]
        idx, mb_c = per_b[b]
        xk = np.zeros((NK, D), bf16)
        xk[:len(idx)] = x[b, g][idx].astype(bf16)
        xq = np.ascontiguousarray(x[b, g][half * NQ:(half + 1) * NQ]).astype(bf16)
        in_maps.append({
            "xq": xq, "xk": xk,
            "xqt": np.ascontiguousarray(xq.T),
            "xkt": np.ascontiguousarray(xk.T),
            "wq_t": wq_t, "wk_t": wk_t, "wv_t": wv_t, "wo_t": wo_t,
            "maskbias": mb_c, "nullk": nk_cols, "nullv_ext": nve,
        })
    return in_maps


def assemble_out(results):
    out = np.zeros((B, G, 2 * NQ, D), np.float32)
    for c in range(8):
        b, g, half = c // 4, (c // 2) % 2, c % 2
        out[b, g, half * NQ:(half + 1) * NQ] = results[c]["out"].astype(np.float32)
    return out


def kernel(**inputs):
    from concourse.bass_utils import run_bass_kernel_spmd

    nc = get_nc()
    in_maps = make_in_maps(**inputs)
    res = run_bass_kernel_spmd(nc, in_maps, core_ids=list(range(8)))
    return assemble_out(res.results)


# revision 34
# speedup vs baseline: 1.0141x; 1.0141x over previous
"""Trainium2 Bass kernel for grouped multi-head attention (nn_Attention_8263517077742).

Reference computation (per batch b, group g, with x [2048, 512]):
  xn   = x / max(||x||_2, eps) * sqrt(512)        (rmsnorm over feature dim)
  q    = (xn * gamma_q) @ wq[g].T                 -> 8 heads of 64
  k,v  = (xn * gamma_c) @ wkv[g].T                -> 8 heads of 64
  null k/v prepended along key sequence; scores masked by mask[b]; softmax;
  merged heads projected by wout[g].

Sharding: 8 cores = 4 (b,g) instances x 2 query-sequence halves. Masked keys
contribute nothing (exp -> 0) and the mask is an input, so the host packs only
the valid key rows (~1010-1040 of 2048) plus one null-kv slot into 9 tiles of
128; pad slots get a -1e30 exp bias. Outputs are disjoint: no collectives.

The PE on this part runs at ~1.2 GHz with a per-matmul weight load, so the
design minimizes PE cycles and keeps the PE streaming:
  - the host supplies x both natural (for the row norms) and pre-transposed
    (bf16 xqt/xkt), so there are no on-chip transposes at all
  - rmsnorm is folded away: projections consume unnormalized xT, and the
    1/||x|| scales are applied as (a) the per-partition `scale` operand of the
    softmax exp for the key side, (b) a broadcast multiply at q staging, (c) a
    per-tile scalar multiply at v staging; gamma, sqrt(D) and the attention
    scale are folded into the weights host-side
  - everything on the PE is bf16 (fp32 PSUM accumulation); fp8 was measured
    and rejected: DoubleRow scores cost 1.65e-2 of the 2e-2 error budget
  - scores are computed transposed (sT [nk, nq]) in 512-query chunks, two
    t-steps ahead of the exp (ScalarE, mask bias + key-norm scale fused)
  - the null k/v pair occupies packed key slot 1120: its k column / v row are
    patched in after the projections, so no separate rank-1 update exists
  - softmax denominators ride as a 65th row of the AV matmul (ones column in
    v); normalization = reciprocal + partition-broadcast + multiply in bf16
  - merged heads are head-pair packed so the output projection contracts over
    full 128 partitions; its per-pair partial sums stream into the attention
    loop as PE fillers and accumulate in SBUF, with per-tile output DMAs
  - k projections lead the prologue (their staging has no alpha dependency)
    interleaved with the norm side-chain; weights/constants load outside the
    timing loop
"""

import sys
from contextlib import ExitStack

import numpy as np
import ml_dtypes

if "/opt/trn_rl_repo" not in sys.path:
    sys.path.insert(0, "/opt/trn_rl_repo")

import concourse.bass as bass  # noqa: E402
import concourse.mybir as mybir  # noqa: E402
from concourse import bacc  # noqa: E402
from concourse.tile import TileContext  # noqa: E402

P = 128
D = 512           # feature dim
E = 512           # inner dim (8 heads x 64)
NQ = 1024         # queries per core
H = 8
DH = 64
NK = 1152         # packed key slots per core (valid keys + pads + null at 1151)
NKT = NK // P     # 9 key tiles
QT = NQ // P      # 8 query tiles
ET = E // P       # 4 e-tiles
DT = D // P       # 4 d-tiles
VEXT = H * (DH + 1)   # 520: per-head v columns + ones column
NULL_SLOT = NK - 32   # 1120: null-kv key slot (partition 96 of the last tile)
F32 = mybir.dt.float32
BF16 = mybir.dt.bfloat16
FP8 = mybir.dt.float8e4
FP8_SCORES = False  # quantize q/k to fp8e4m3, score matmuls in DoubleRow mode

B, G = 2, 2


def build_nc(reps=1):
    nc = bacc.Bacc(
        trn_type="TRN2",
        target_bir_lowering=False,
        debug=False,
        enable_asserts=False,
        num_devices=8,
    )
    xq_ext = nc.declare_dram_parameter("xq", [NQ, D], BF16, isOutput=False)
    xk_ext = nc.declare_dram_parameter("xk", [NK, D], BF16, isOutput=False)
    xqt_ext = nc.declare_dram_parameter("xqt", [D, NQ], BF16, isOutput=False)
    xkt_ext = nc.declare_dram_parameter("xkt", [D, NK], BF16, isOutput=False)
    wq_ext = nc.declare_dram_parameter("wq_t", [D, E], BF16, isOutput=False)
    wk_ext = nc.declare_dram_parameter("wk_t", [D, E], BF16, isOutput=False)
    wv_ext = nc.declare_dram_parameter("wv_t", [D, E], BF16, isOutput=False)
    wo_ext = nc.declare_dram_parameter("wo_t", [E, D], BF16, isOutput=False)
    mb_ext = nc.declare_dram_parameter("maskbias", [P, NKT], F32, isOutput=False)
    nk_ext = nc.declare_dram_parameter("nullk", [P, ET],
                                       FP8 if FP8_SCORES else BF16, isOutput=False)
    nv_ext = nc.declare_dram_parameter("nullv_ext", [1, VEXT], BF16, isOutput=False)
    out_ext = nc.declare_dram_parameter("out", [NQ, D], BF16, isOutput=True)

    with TileContext(nc) as tc, ExitStack() as ctx:
        # ---- persistent SBUF tiles (weights/consts loaded outside the loop) ----
        persist = ctx.enter_context(tc.tile_pool(name="persist", bufs=1))
        # transposed normalized x, dj-blocked single tiles
        xqT = persist.tile([P, DT * NQ], BF16, name="xqT", tag="xqT")
        xkT = persist.tile([P, DT * NK], BF16, name="xkT", tag="xkT")
        QKDT = FP8 if FP8_SCORES else BF16
        qT = [persist.tile([P, NQ], QKDT, name=f"qT{j}", tag=f"qT{j}") for j in range(ET)]
        kT = [persist.tile([P, NK], QKDT, name=f"kT{j}", tag=f"kT{j}") for j in range(ET)]
        if FP8_SCORES:
            # partition-folded DoubleRow operands: [32, (head, plane, n)]
            q8 = [persist.tile([32, 4 * NQ], FP8, name=f"q8{j}", tag=f"q8{j}")
                  for j in range(ET)]
            k8 = [persist.tile([32, 4 * NK], FP8, name=f"k8{j}", tag=f"k8{j}")
                  for j in range(ET)]
        v_ext = [persist.tile([P, VEXT], BF16, name=f"vx{i}", tag=f"vx{i}") for i in range(NKT)]
        mgT = [persist.tile([P, NQ], BF16, name=f"mg{p}", tag=f"mg{p}") for p in range(H // 2)]
        wq_sb = [persist.tile([P, E], BF16, name=f"wq{j}", tag=f"wq{j}") for j in range(DT)]
        wk_sb = [persist.tile([P, E], BF16, name=f"wk{j}", tag=f"wk{j}") for j in range(DT)]
        wv_sb = [persist.tile([P, E], BF16, name=f"wv{j}", tag=f"wv{j}") for j in range(DT)]
        wo_sb = [persist.tile([P, D], BF16, name=f"wo{p}", tag=f"wo{p}") for p in range(H // 2)]
        mb_sb = persist.tile([P, NKT], F32, name="mb", tag="mb")
        aq_sb = persist.tile([P, QT], F32, name="aq", tag="aq")
        ak_sb = persist.tile([P, NKT], F32, name="ak", tag="ak")
        sk_sb = persist.tile([P, NKT], F32, name="sk", tag="sk")
        arow = persist.tile([1, NQ], F32, name="arow", tag="arow")
        abcq = persist.tile([P, NQ], F32, name="abcq", tag="abcq")
        nk_sb = persist.tile([P, ET], FP8 if FP8_SCORES else BF16, name="nk", tag="nk")
        nv_sb = persist.tile([1, VEXT], BF16, name="nv", tag="nv")
        onesc = persist.tile([P, H], BF16, name="onesc", tag="onesc")
        osb = persist.tile([P, QT * D], BF16, name="osb", tag="osb")

        # ---- PSUM pools (2 + 4 + 2 = 8 banks) ----
        ppsum = ctx.enter_context(tc.tile_pool(name="ppsum", bufs=2, space="PSUM"))
        sps = ctx.enter_context(tc.tile_pool(name="sps", bufs=2, space="PSUM"))
        avps = ctx.enter_context(tc.tile_pool(name="avps", bufs=1, space="PSUM"))

        # ---- working SBUF pools ----
        xpool = ctx.enter_context(tc.tile_pool(name="xpool", bufs=6))
        xnpool = ctx.enter_context(tc.tile_pool(name="xnpool", bufs=3))
        ppool = ctx.enter_context(tc.tile_pool(name="ppool", bufs=3))
        rpool = ctx.enter_context(tc.tile_pool(name="rpool", bufs=2))

        nc.gpsimd.memset(onesc[:, :], 1.0)
        nc.sync.dma_start(out=mb_sb[:, :], in_=mb_ext[:, :])
        nc.sync.dma_start(out=nk_sb[:, :], in_=nk_ext[:, :])
        nc.sync.dma_start(out=nv_sb[:, :], in_=nv_ext[:, :])
        for j in range(DT):
            nc.sync.dma_start(out=wq_sb[j][:, :], in_=wq_ext[j * P:(j + 1) * P, :])
            nc.sync.dma_start(out=wk_sb[j][:, :], in_=wk_ext[j * P:(j + 1) * P, :])
            nc.sync.dma_start(out=wv_sb[j][:, :], in_=wv_ext[j * P:(j + 1) * P, :])
        for p in range(H // 2):
            nc.sync.dma_start(out=wo_sb[p][:, :], in_=wo_ext[p * P:(p + 1) * P, :])

        if reps > 1:
            ctx.enter_context(tc.For_i(
                0, reps, 1,
                hint_engines=(
                    mybir.EngineType.PE, mybir.EngineType.DVE,
                    mybir.EngineType.Activation, mybir.EngineType.SP,
                    mybir.EngineType.Pool,
                ),
            ))

        # per-rep DMAs: host-transposed x feeds the projections directly;
        # xkt first -- the k projection is the PE's first work of the rep
        for dj in range(DT):
            nc.sync.dma_start(out=xkT[:, dj * NK:(dj + 1) * NK],
                              in_=xkt_ext[dj * P:(dj + 1) * P, :])
        for dj in range(DT):
            nc.sync.dma_start(out=xqT[:, dj * NQ:(dj + 1) * NQ],
                              in_=xqt_ext[dj * P:(dj + 1) * P, :])
        xts = {}

        def dma_x(kind, i):
            xt = xpool.tile([P, D], BF16, name="x", tag="x", bufs=6)
            src = xq_ext if kind == "q" else xk_ext
            nc.sync.dma_start(out=xt[:, :], in_=src[i * P:(i + 1) * P, :])
            xts[(kind, i)] = xt

        for i in range(QT):
            dma_x("q", i)
        for i in range(NKT):
            dma_x("k", i)

        def norm_alpha(kind, i):
            """rmsnorm scale alpha[i] = 1/max(||x_row||, eps) for one tile."""
            xt = xts[(kind, i)]
            xsq = xnpool.tile([P, D], BF16, name="xsq", tag="xsq")
            ss = xnpool.tile([P, 1], F32, name="ss", tag="ss")
            nc.gpsimd.tensor_mul(xsq[:, :], xt[:, :], xt[:, :])
            nc.vector.tensor_reduce(
                ss[:, :], xsq[:, :], axis=mybir.AxisListType.X,
                op=mybir.AluOpType.add,
            )
            nrm = xnpool.tile([P, 1], F32, name="nrm", tag="nrm")
            nc.scalar.activation(
                nrm[:, :], ss[:, :], mybir.ActivationFunctionType.Sqrt,
            )
            nc.gpsimd.tensor_scalar_max(nrm[:, :], nrm[:, :], 1e-12)
            a_sb = aq_sb if kind == "q" else ak_sb
            nc.vector.reciprocal(a_sb[:, i:i + 1], nrm[:, :])

        def emit_qproj(c):
            """qT[:, c*512:(c+1)*512] for all 4 e-tiles."""
            for j in range(ET):
                pq = ppsum.tile([P, D], F32, name="pp", tag="pp")
                for dj in range(DT):
                    nc.tensor.matmul(
                        pq[:, :],
                        lhsT=wq_sb[dj][:, j * P:(j + 1) * P],
                        rhs=xqT[:, dj * NQ + c * 512:dj * NQ + (c + 1) * 512],
                        start=(dj == 0), stop=(dj == DT - 1),
                    )
                nc.vector.tensor_mul(qT[j][:, c * 512:(c + 1) * 512],
                                     pq[:, :], abcq[:, c * 512:(c + 1) * 512])
                if FP8_SCORES and c == 1:
                    for hh in range(2):
                        for two in range(2):
                            nc.sync.dma_start(
                                out=q8[j][:, (hh * 2 + two) * NQ:
                                          (hh * 2 + two + 1) * NQ],
                                in_=qT[j][hh * DH + two * 32:
                                          hh * DH + two * 32 + 32, :],
                            )

        KCH = [(0, 512), (512, 1024), (1024, NK)]   # k-proj column chunks

        def emit_kproj(j, c):
            lo, hi = KCH[c]
            pk = ppsum.tile([P, D], F32, name="pp", tag="pp")
            for dj in range(DT):
                nc.tensor.matmul(
                    pk[:, 0:hi - lo],
                    lhsT=wk_sb[dj][:, j * P:(j + 1) * P],
                    rhs=xkT[:, dj * NK + lo:dj * NK + hi],
                    start=(dj == 0), stop=(dj == DT - 1),
                )
            nc.vector.tensor_copy(kT[j][:, lo:hi], pk[:, 0:hi - lo])
            if c == len(KCH) - 1:
                # null-k column: its packed key slot bypasses the projection
                nc.gpsimd.tensor_copy(kT[j][:, NULL_SLOT:NULL_SLOT + 1],
                                      nk_sb[:, j:j + 1])
                if FP8_SCORES:
                    for hh in range(2):
                        for two in range(2):
                            nc.sync.dma_start(
                                out=k8[j][:, (hh * 2 + two) * NK:
                                          (hh * 2 + two + 1) * NK],
                                in_=kT[j][hh * DH + two * 32:
                                          hh * DH + two * 32 + 32, :],
                            )

        def emit_vproj(i):
            pv = ppsum.tile([P, D], F32, name="pp", tag="pp")
            for dj in range(DT):
                nc.tensor.matmul(
                    pv[:, :],
                    lhsT=xkT[:, dj * NK + i * P:dj * NK + (i + 1) * P],
                    rhs=wv_sb[dj][:, :],
                    start=(dj == 0), stop=(dj == DT - 1),
                )
            src = pv[:, :].rearrange("p (a d) -> p a d", a=H)
            dst = v_ext[i][:, :].rearrange("p (a r) -> p a r", a=H)
            nc.vector.tensor_scalar(dst[:, :, 0:DH], src[:, :, :],
                                    ak_sb[:, i:i + 1], None,
                                    op0=mybir.AluOpType.mult)
            nc.gpsimd.tensor_copy(dst[:, :, DH:DH + 1],
                                  onesc[:, :].rearrange("p (a r) -> p a r", a=H))
            if i == NKT - 1:
                # null-v row (+ its ones entry) at the null key slot
                p0 = NULL_SLOT % P
                nc.gpsimd.tensor_copy(v_ext[i][p0:p0 + 1, :], nv_sb[:, :])

        # ---- prologue ----
        # k projections lead: their staging copies have no alpha dependency,
        # so PE/DVE stream from the moment xkt lands; the norm side-chain
        # (Pool mul/reduce/max + ACT sqrt + DVE recip) fills in behind
        emit_kproj(0, 0)
        emit_kproj(0, 1)
        emit_kproj(0, 2)
        # interleave the remaining k-proj chunks with the norm side-chain so
        # neither DVE staging nor the alpha chain head-of-line blocks the PE
        norms = [("q", i) for i in range(QT)] + [("k", i) for i in range(NKT)]
        kchunks = [(j, c) for j in range(1, ET) for c in range(len(KCH))]
        nk_ratio = [1] * 6 + [2] * 6
        while norms or kchunks:
            if kchunks:
                j, c = kchunks.pop(0)
                emit_kproj(j, c)
            for _ in range(nk_ratio.pop(0) if nk_ratio else 2):
                if norms:
                    norm_alpha(*norms.pop(0))
        # query-side alpha: [128, QT] -> one [1, NQ] row -> partition bcast
        for t in range(QT):
            nc.sync.dma_start(out=arow[:, t * P:(t + 1) * P],
                              in_=aq_sb[:, t:t + 1])
        nc.gpsimd.partition_broadcast(abcq[:, :], arow[:, :])
        # key-side alpha becomes the exp scale; null slot must not be scaled
        nc.gpsimd.tensor_copy(sk_sb[:, :], ak_sb[:, :])
        nc.gpsimd.memset(sk_sb[NULL_SLOT % P:NULL_SLOT % P + 1,
                               NULL_SLOT // P:NULL_SLOT // P + 1], 1.0)

        emit_qproj(0)
        emit_qproj(1)
        for i in range(NKT):
            emit_vproj(i)
        # dummy exp: pulls the exp table-set load off the first score's path
        escr = xnpool.tile([1, 1], F32, name="escr", tag="escr")
        nc.scalar.activation(escr[:, :], mb_sb[0:1, 0:1],
                             mybir.ActivationFunctionType.Exp)

        # filler queue: streamed output-projection units join per head pair
        fillers = []

        # ---- attention: 9 key tiles x 8 heads, scores 2 t-steps ahead ----
        def emit_scores(h, t):
            j, hh = h // 2, h % 2
            st = sps.tile([P, NQ], F32, name="st", tag="st")
            if FP8_SCORES:
                kv_ = k8[j][:, :].rearrange("p (hh two n) -> p hh two n", hh=2, two=2)
                qv_ = q8[j][:, :].rearrange("p (hh two n) -> p hh two n", hh=2, two=2)
                for c in range(2):
                    nc.tensor.matmul(
                        st[:, c * 512:(c + 1) * 512],
                        lhsT=kv_[:, hh, :, t * P:(t + 1) * P],
                        rhs=qv_[:, hh, :, c * 512:(c + 1) * 512],
                        start=True, stop=True,
                        perf_mode=mybir.MatmulPerfMode.DoubleRow,
                    )
            else:
                off = DH * hh
                for c in range(2):
                    nc.tensor.matmul(
                        st[:, c * 512:(c + 1) * 512],
                        lhsT=kT[j][off:off + DH, t * P:(t + 1) * P],
                        rhs=qT[j][off:off + DH, c * 512:(c + 1) * 512],
                        start=True, stop=True,
                    )
            return st

        def emit_oproj(p, cq):
            po = ppsum.tile([P, D], F32, name="pp", tag="pp")
            nc.tensor.matmul(
                po[:, :],
                lhsT=mgT[p][:, cq * P:(cq + 1) * P],
                rhs=wo_sb[p][:, :],
                start=True, stop=True,
            )
            if p == 0:
                nc.vector.tensor_copy(osb[:, cq * D:(cq + 1) * D], po[:, :])
            else:
                nc.vector.tensor_add(osb[:, cq * D:(cq + 1) * D],
                                     osb[:, cq * D:(cq + 1) * D], po[:, :])
            if p == H // 2 - 1:
                nc.sync.dma_start(out=out_ext[cq * P:(cq + 1) * P, :],
                                  in_=osb[:, cq * D:(cq + 1) * D])

        pending = []      # queue of (h, t, [st chunks]) not yet exp'd
        av = None

        def emit_step():
            """exp + AV for the oldest pending score pair."""
            h, t, st = pending.pop(0)
            pt = ppool.tile([P, NQ], BF16, name="pt", tag="pt")
            nc.scalar.activation(
                pt[:, :], st[:, :], mybir.ActivationFunctionType.Exp,
                bias=mb_sb[:, t:t + 1], scale=sk_sb[:, t:t + 1],
            )
            for c in range(2):
                nc.tensor.matmul(
                    av[:, c * 512:(c + 1) * 512],
                    lhsT=v_ext[t][:, h * (DH + 1):(h + 1) * (DH + 1)],
                    rhs=pt[:, c * 512:(c + 1) * 512],
                    start=(t == 0), stop=(t == NKT - 1),
                )

        for h in range(H):
            av = avps.tile([DH + 1, NQ], F32, name="av", tag="av")
            if h == 0:
                pending.append((0, 0, emit_scores(0, 0)))
                pending.append((0, 1, emit_scores(0, 1)))
            for t in range(NKT):
                # keep scores 2 steps ahead of exp/AV
                if t + 2 < NKT:
                    pending.append((h, t + 2, emit_scores(h, t + 2)))
                elif h + 1 < H:
                    t2 = t + 2 - NKT
                    pending.append((h + 1, t2, emit_scores(h + 1, t2)))
                emit_step()
                if fillers:
                    fillers.pop()()
            # normalize head h: avc (DVE) frees the PSUM bank, then
            # recip (DVE) + broadcast (Pool) + merge mul (DVE), all bf16;
            # the last head runs in halves so pair-3 out-proj starts sooner
            halves = ((0, 512), (512, NQ)) if h == H - 1 else ((0, NQ),)
            avc = rpool.tile([DH + 1, NQ], BF16, name="avc", tag="avc")
            recip = rpool.tile([1, NQ], BF16, name="recip", tag="recip")
            rbc = rpool.tile([DH, NQ], BF16, name="rbc", tag="rbc")
            off = DH * (h % 2)
            for lo, hi in halves:
                nc.vector.tensor_copy(avc[:, lo:hi], av[:, lo:hi])
                with nc.allow_low_precision(reason="bf16 softmax renorm; tol 2e-2"):
                    nc.vector.reciprocal(recip[:, lo:hi], avc[DH:DH + 1, lo:hi])
                nc.gpsimd.partition_broadcast(rbc[:, lo:hi], recip[:, lo:hi])
                nc.vector.tensor_mul(mgT[h // 2][off:off + DH, lo:hi],
                                     avc[0:DH, lo:hi], rbc[:, lo:hi])
                if h == H - 1:
                    for cq in range(lo // P, hi // P):
                        emit_oproj(h // 2, cq)
            if h % 2 == 1 and h < H - 1:
                pr = h // 2
                fillers[0:0] = [lambda p=pr, cq=cq: emit_oproj(p, cq)
                                for cq in reversed(range(QT))]

        # ---- drain remaining output-projection units (DMAs stream per cq) ----
        while fillers:
            fillers.pop()()

    nc.compile()
    return nc


_NC_CACHE = []


def get_nc():
    if not _NC_CACHE:
        _NC_CACHE.append(build_nc())
    return _NC_CACHE[0]


def make_in_maps(x, mask, gamma_q, gamma_c, wq, wkv, wout, null_kv):
    x = np.asarray(x, dtype=np.float32)
    mask = np.asarray(mask)
    gamma_q = np.asarray(gamma_q, dtype=np.float32)
    gamma_c = np.asarray(gamma_c, dtype=np.float32)
    wq = np.asarray(wq, dtype=np.float32)
    wkv = np.asarray(wkv, dtype=np.float32)
    wout = np.asarray(wout, dtype=np.float32)
    null_kv = np.asarray(null_kv, dtype=np.float32)
    bf16 = ml_dtypes.bfloat16

    sqD = np.float32(np.sqrt(D))
    scale = np.float32(DH ** -0.5)
    DI = E

    per_g = {}
    for g in range(G):
        wq_t = np.ascontiguousarray(
            (wq[g] * (gamma_q[g] * sqD * scale)[None, :]).T).astype(bf16)
        wk_t = np.ascontiguousarray(
            (wkv[g][:DI] * (gamma_c[g] * sqD)[None, :]).T).astype(bf16)
        wv_t = np.ascontiguousarray(
            (wkv[g][DI:] * (gamma_c[g] * sqD)[None, :]).T).astype(bf16)
        wo_t = np.ascontiguousarray(wout[g].T).astype(bf16)
        # null-k stacked e-major -> [128, 4] columns; null-v interleaved + ones
        nk_flat = null_kv[0, g].reshape(E)             # [H,1,DH] -> [512]
        nk_dt = ml_dtypes.float8_e4m3 if FP8_SCORES else bf16
        nk_cols = np.ascontiguousarray(nk_flat.reshape(ET, P).T).astype(nk_dt)
        nve = np.zeros((1, VEXT), np.float32)
        for h in range(H):
            nve[0, h * (DH + 1):h * (DH + 1) + DH] = null_kv[1, g, h, 0, :]
            nve[0, h * (DH + 1) + DH] = 1.0
        per_g[g] = (wq_t, wk_t, wv_t, wo_t, nk_cols, nve.astype(bf16))

    per_b = {}
    for b in range(B):
        idx = np.nonzero(mask[b])[0]
        nv = len(idx)
        assert nv <= NULL_SLOT, f"valid keys {nv} exceed packed capacity {NULL_SLOT}"
        bias = np.full(NK, np.float32(-1e30), np.float32)
        bias[:nv] = 0.0
        bias[NULL_SLOT] = 0.0               # null slot is always valid
        per_b[b] = (idx, np.ascontiguousarray(bias.reshape(NKT, P).T))

    in_maps = []
    for c in range(8):
        b, g, half = c // 4, (c // 2) % 2, c % 2
        wq_t, wk_t, wv_t, wo_t, nk_cols, nve = per_g[g]
        idx, mb_c = per_b[b]
        xk = np.zeros((NK, D), bf16)
        xk[:len(idx)] = x[b, g][idx].astype(bf16)
        xq = np.ascontiguousarray(x[b, g][half * NQ:(half + 1) * NQ]).astype(bf16)
        in_maps.append({
            "xq": xq, "xk": xk,
            "xqt": np.ascontiguousarray(xq.T),
            "xkt": np.ascontiguousarray(xk.T),
            "wq_t": wq_t, "wk_t": wk_t, "wv_t": wv_t, "wo_t": wo_t,
            "maskbias": mb_c, "nullk": nk_cols, "nullv_ext": nve,
        })
    return in_maps


def assemble_out(results):
    out = np.zeros((B, G, 2 * NQ, D), np.float32)
    for c in range(8):
        b, g, half = c // 4, (c // 2) % 2, c % 2
        out[b, g, half * NQ:(half + 1) * NQ] = results[c]["out"].astype(np.float32)
    return out


def kernel(**inputs):
    from concourse.bass_utils import run_bass_kernel_spmd

    nc = get_nc()
    in_maps = make_in_maps(**inputs)
    res = run_bass_kernel_spmd(nc, in_maps, core_ids=list(range(8)))
    return assemble_out(res.results)
